# revision 19
# baseline (speedup 1.0000x reference)
"""Trainium2 Bass kernel for nn_BaseMultiHeadAttention (B=2, S=2048, E=1024, H=16).

Sharding: tensor-parallel over heads — each of the 8 NeuronCores handles 2
heads for both batch elements (4 (b,h) jobs/core).  RMSNorm + RoPE + causal
attention run per-head on-device; the output projection is row-sharded
(each core contracts its 128 ctx features against proj_w), and the host
sums the 8 partial [B,S,E] outputs (the all-reduce) and adds the bias.

Device pipeline per core:
  Phase A (per job): load q/k [128, NT*64] (host pre-arranged
    partition-major so every DMA descriptor is a 4KB contiguous run),
    sum-of-squares -> sqrt(mean+eps) on ACT -> reciprocal on DVE, then
    batched RoPE (host permutes q/k features to de-interleave the rope
    pairs — a consistent permutation leaves q.k dot products unchanged,
    so every rope op is a contiguous full-width DVE op), then
    PE-transpose to qT/kT [64, S] (float32r) for the attention matmuls.
  Phase B (per b, q-chunk of 512, head): scoresT[k,q] = kT.T @ qT on PE in
    float32r (1 cyc/row), causal block-sparse (only k-blocks <= chunk end),
    triangular mask add on diagonal blocks (DVE), exp via ACT over
    [128,1024] PSUM groups (scale=D^-0.5 folded in; no max subtraction:
    RMSNorm bounds |scores*scale| <= ~8), ctx = p.T @ [v|1] accumulated in
    PSUM (the ones column yields softmax row-sums for free), rows scaled by
    1/sum on the PSUM->SBUF copy.  After both heads: PE-transpose ctx
    [s,128f] -> [128f,s], partial projection in float32r, DMA out.
  A and B share PSUM pools and are interleaved A0 A1 B0 A2 A3 B1 so the
  scheduler overlaps phase A of later jobs under phase B compute.
"""
import numpy as np

import bass_rust
import concourse.bass as bass
import concourse.mybir as mybir
import concourse.tile as tile
from concourse.bass_utils import run_bass_kernel_spmd
from concourse.masks import make_identity

B, S, E, H, D = 2, 2048, 1024, 16, 64
HD = D // 2
N_CORES = 8
HL = H // N_CORES          # 2 heads per core
NJ = B * HL                # 4 (b, h) jobs per core
NT = S // 128              # 16 s-tiles per job
NCH = S // 512             # 4 q-chunks per job
EPS = 1.1920928955078125e-07
SCALE = float(D) ** -0.5
NEG = -1e30
f32 = mybir.dt.float32
f32r = mybir.dt.float32r
ALU = mybir.AluOpType
ACTF = mybir.ActivationFunctionType

# float32r runs the PE at 1 cycle/row for moving dims >= 256 (vs fp32's 4).
F32R_SCORES = True
F32R_PROJ = True
# bf16 for the attention-weights @ v matmul (PE 1 cyc/row at N=65)
CTX_BF16 = False
bf16 = mybir.dt.bfloat16

_TC = tile.TileContext


def _legalize_waits(nc):
    """Split multi-wait sync_infos for this walrus build.

    This neuronxcc's codegen allows 1 sync wait per instruction (2 on
    EventSemaphore), while the Tile scheduler attaches all outstanding
    waits to one instruction.  Hoist the excess onto same-engine NoOps
    inserted immediately before the offending instruction — the engine
    executes its stream in order, so blocking semantics are identical.
    """
    uid = 0
    for f in nc.m.functions:
        for blk in f.blocks:
            insts = list(blk.instructions)
            out, changed = [], False
            for inst in insts:
                si = inst.sync_info
                cap = 2 if isinstance(inst, mybir.InstEventSemaphore) else 1
                if si is not None and len(si.on_wait) > cap:
                    changed = True
                    waits = list(si.on_wait)
                    for w in waits[:-cap]:
                        carrier = mybir.InstNoOp(
                            name=f"legwait-{uid}", engine=inst.engine,
                            ins=[], outs=[])
                        uid += 1
                        carrier.sync_info = bass_rust.SyncInfo(
                            on_wait=[w], on_update=[])
                        nc.register_instruction(carrier, overwrite=True)
                        out.append(carrier)
                    si.on_wait = waits[-cap:]
                    inst.sync_info = si
                out.append(inst)
            if changed:
                blk.instructions = out


def build_nc():
    nc = bass.Bass("TRN2", target_bir_lowering=False, debug=False)
    q_in = nc.dram_tensor("q", [NJ, 128, NT, D], f32, kind="ExternalInput")
    k_in = nc.dram_tensor("k", [NJ, 128, NT, D], f32, kind="ExternalInput")
    v_in = nc.dram_tensor("v", [NJ, 128, NT, D], f32, kind="ExternalInput")
    cos_in = nc.dram_tensor("cos", [128, NT, HD], f32, kind="ExternalInput")
    sin_in = nc.dram_tensor("sin", [128, NT, HD], f32, kind="ExternalInput")
    wt_in = nc.dram_tensor("wt", [128, E], f32, kind="ExternalInput")
    out = nc.dram_tensor("out", [B * S, E], f32, kind="ExternalOutput")

    with _TC(nc) as tc:
        with tc.tile_pool(name="const", bufs=1) as cp, \
             tc.tile_pool(name="pa", bufs=2) as pa, \
             tc.tile_pool(name="pb", bufs=2) as pb, \
             tc.tile_pool(name="pp", bufs=12) as pp, \
             tc.tile_pool(name="po", bufs=4) as po, \
             tc.tile_pool(name="ps_s", bufs=2, space="PSUM") as ps_s, \
             tc.tile_pool(name="ps_sm", bufs=2, space="PSUM") as ps_sm, \
             tc.tile_pool(name="ps_o", bufs=2, space="PSUM") as ps_o:
            ident = cp.tile([128, 128], f32)
            make_identity(nc, ident)
            eps_t = cp.tile([128, 1], f32)
            nc.vector.memset(eps_t, EPS)
            cos_sb = cp.tile([128, NT, HD], f32)
            sin_sb = cp.tile([128, NT, HD], f32)
            nc.sync.dma_start(out=cos_sb, in_=cos_in.ap())
            nc.sync.dma_start(out=sin_sb, in_=sin_in.ap())
            wt_sb = cp.tile([128, E], f32r if F32R_PROJ else f32)
            wt_raw = cp.tile([128, E], f32)
            nc.sync.dma_start(out=wt_raw, in_=wt_in.ap())
            nc.vector.tensor_copy(wt_sb, wt_raw)
            qT = cp.tile([64, NJ, S], f32r if F32R_SCORES else f32)
            kT = cp.tile([64, NJ, S], f32r if F32R_SCORES else f32)
            vsb = cp.tile([128, NJ, NT, D + 1],
                          bf16 if CTX_BF16 else f32)

            # ------------- Phase A: norm + rope + transpose (per job) -------
            def phase_a(j):
                nc.gpsimd.memset(vsb[:, j, :, :], 1.0)
                if CTX_BF16:
                    vraw = pa.tile([128, NT, D], f32, tag="vraw", name="vraw")
                    nc.sync.dma_start(out=vraw, in_=v_in.ap()[j])
                    nc.vector.tensor_copy(vsb[:, j, :, 0:D], vraw)
                else:
                    nc.sync.dma_start(out=vsb[:, j, :, 0:D], in_=v_in.ap()[j])
                for dstT, src in ((qT, q_in), (kT, k_in)):
                    raw = pa.tile([128, NT, D], f32, tag="raw", name="raw")
                    nc.sync.dma_start(out=raw, in_=src.ap()[j])
                    sq = pa.tile([128, NT, D], f32, tag="sq", name="sq")
                    nc.any.tensor_mul(sq, raw, raw)
                    ss = pa.tile([128, NT], f32, tag="ss", name="ss")
                    nc.vector.reduce_sum(ss, sq, axis=mybir.AxisListType.X)
                    rs = pa.tile([128, NT], f32, tag="rs", name="rs")
                    nc.scalar.activation(
                        out=rs, in_=ss, func=ACTF.Sqrt,
                        bias=eps_t, scale=1.0 / D,
                    )
                    nc.vector.reciprocal(out=rs, in_=rs)
                    # normalize: raw * rs (rs broadcast over d via step-0 AP)
                    rs_b = bass.AP(
                        tensor=rs.tensor, offset=rs.offset,
                        ap=[list(rs.ap[0]), list(rs.ap[1]), [0, D]])
                    xn = pa.tile([128, NT, D], f32, tag="xn", name="xn")
                    nc.gpsimd.tensor_mul(xn, raw, rs_b)
                    x1, x2 = xn[:, :, 0:HD], xn[:, :, HD:D]
                    rn_all = pa.tile([128, NT, D], f32, tag="rn", name="rn")
                    t1 = pa.tile([128, NT, HD], f32, tag="t1", name="t1")
                    t2 = pa.tile([128, NT, HD], f32, tag="t2", name="t2")
                    nc.any.tensor_mul(t1, x1, cos_sb)
                    nc.any.tensor_mul(t2, x2, sin_sb)
                    nc.any.tensor_sub(rn_all[:, :, 0:HD], t1, t2)
                    nc.any.tensor_mul(t1, x1, sin_sb)
                    nc.any.tensor_mul(t2, x2, cos_sb)
                    nc.any.tensor_add(rn_all[:, :, HD:D], t1, t2)
                    for tg in range(NT // 4):
                        ps_tr = ps_o.tile([64, 512], f32, tag="o",
                                          name="ps_tr")
                        for tt in range(4):
                            t = tg * 4 + tt
                            nc.tensor.transpose(
                                ps_tr[:, tt * 128:(tt + 1) * 128],
                                rn_all[:, t, :], ident)
                        nc.any.tensor_copy(
                            dstT[:, j, tg * 512:(tg + 1) * 512], ps_tr)

            # ------------- Phase B: attention + projection (per batch) ------
            def phase_b(b):
                for c in range(NCH):
                    cpair = [pb.tile([128, HL * D], f32, tag=f"cpair{qb}",
                                     name=f"cpair{qb}")
                             for qb in range(4)]
                    for hl in range(HL):
                        j = b * HL + hl
                        ngrp = 2 * c + 2
                        ptiles = []
                        for g in range(ngrp):
                            sps = ps_s.tile([128, 1024], f32, tag="s",
                                            name="sps")
                            for u in range(2):
                                jj = 2 * g + u
                                nc.tensor.matmul(
                                    sps[:, u * 512:(u + 1) * 512],
                                    lhsT=kT[:, j, jj * 128:(jj + 1) * 128],
                                    rhs=qT[:, j, c * 512:(c + 1) * 512],
                                    start=True, stop=True,
                                )
                            pt = pp.tile([128, 1024],
                                         bf16 if CTX_BF16 else f32,
                                         tag="p", name="pt")
                            nc.scalar.activation(
                                out=pt, in_=sps, func=ACTF.Exp, scale=SCALE)
                            for u in range(2):
                                jj = 2 * g + u
                                if jj >= 4 * c:
                                    # diag block: zero the non-causal (q < k)
                                    # part of p on the idle gpsimd engine
                                    tl = jj - 4 * c
                                    sl = pt[:, u * 512 + tl * 128:
                                            u * 512 + (tl + 1) * 128]
                                    nc.gpsimd.affine_select(
                                        out=sl, in_=sl, compare_op=ALU.is_ge,
                                        fill=0.0, base=0, pattern=[[1, 128]],
                                        channel_multiplier=-1)
                            ptiles.append(pt)
                        ctx = ps_sm.tile([128, 4, D + 1], f32, tag="sm",
                                         name="ctx")
                        for qb in range(4):
                            i = 4 * c + qb
                            for jj in range(i + 1):
                                nc.tensor.matmul(
                                    ctx[:, qb, :],
                                    lhsT=ptiles[jj // 2][
                                        :, (jj % 2) * 512 + qb * 128:
                                           (jj % 2) * 512 + (qb + 1) * 128],
                                    rhs=vsb[:, j, jj, :],
                                    start=(jj == 0), stop=(jj == i),
                                )
                            rsum = pb.tile([128, 1], f32, tag="rsum",
                                           name="rsum")
                            nc.vector.reciprocal(out=rsum,
                                                 in_=ctx[:, qb, D:D + 1])
                            nc.vector.tensor_scalar_mul(
                                cpair[qb][:, hl * D:(hl + 1) * D],
                                ctx[:, qb, 0:D], rsum)
                    for qb in range(4):
                        i = 4 * c + qb
                        trp = ps_sm.tile([128, 128], f32, tag="sm", name="trp")
                        nc.tensor.transpose(trp, cpair[qb], ident)
                        ctxT = pb.tile([128, 128], f32r if F32R_PROJ else f32,
                                       tag="ctxT", name="ctxT")
                        nc.any.tensor_copy(ctxT, trp)
                        osb = po.tile([128, E], f32, tag="osb", name="osb")
                        for n in range(2):
                            ops_ = ps_o.tile([128, 512], f32, tag="o",
                                             name="ops")
                            nc.tensor.matmul(
                                ops_, lhsT=ctxT,
                                rhs=wt_sb[:, n * 512:(n + 1) * 512],
                                start=True, stop=True,
                            )
                            nc.any.tensor_copy(
                                osb[:, n * 512:(n + 1) * 512], ops_)
                        nc.gpsimd.dma_start(
                            out=out.ap()[b * S + i * 128:
                                         b * S + (i + 1) * 128, :],
                            in_=osb,
                        )

            phase_a(0)
            phase_a(1)
            phase_b(0)
            phase_a(2)
            phase_a(3)
            phase_b(1)
    _legalize_waits(nc)
    return nc


# even rope lanes first, then odd — see Phase A comment
_ROPE_PERM = np.concatenate([np.arange(0, D, 2), np.arange(1, D, 2)])


def _shard_inputs(q, k, v, cos, sin, proj_w):
    """Per-core input maps (host-side layout prep only — no module math)."""
    qh = q.reshape(B, S, H, D)
    kh = k.reshape(B, S, H, D)
    vh = v.reshape(B, S, H, D)
    # [S, HD] -> [128, NT, HD] partition-major
    cos_t = np.ascontiguousarray(
        cos.reshape(NT, 128, HD).transpose(1, 0, 2), np.float32)
    sin_t = np.ascontiguousarray(
        sin.reshape(NT, 128, HD).transpose(1, 0, 2), np.float32)
    maps = []
    for core in range(N_CORES):
        hs = slice(HL * core, HL * (core + 1))

        def tiles(x, permute):
            xs = x[:, :, hs, :].transpose(0, 2, 1, 3)  # [B, HL, S, D]
            if permute:
                xs = xs[..., _ROPE_PERM]
            # [NJ, NT, 128, D] -> [NJ, 128, NT, D] partition-major
            return np.ascontiguousarray(
                xs.reshape(NJ, NT, 128, D).transpose(0, 2, 1, 3), np.float32)

        wt_c = np.ascontiguousarray(
            proj_w[:, 128 * core:128 * (core + 1)].T, np.float32)
        maps.append({
            "q": tiles(qh, True), "k": tiles(kh, True),
            "v": tiles(vh, False),
            "cos": cos_t, "sin": sin_t, "wt": wt_c,
        })
    return maps


_NC_CACHE = []


def _get_nc():
    if not _NC_CACHE:
        _NC_CACHE.append(build_nc())
    return _NC_CACHE[0]


def kernel(q, k, v, attn_mask, padding_mask, qn_w, kn_w, proj_w, proj_b,
           cos, sin):
    q = np.asarray(q, np.float32)
    k = np.asarray(k, np.float32)
    v = np.asarray(v, np.float32)
    proj_w = np.asarray(proj_w, np.float32)
    proj_b = np.asarray(proj_b, np.float32)
    cos = np.asarray(cos, np.float32)
    sin = np.asarray(sin, np.float32)
    attn_mask = np.asarray(attn_mask)
    padding_mask = np.asarray(padding_mask)
    qn_w = np.asarray(qn_w, np.float32)
    kn_w = np.asarray(kn_w, np.float32)
    # The kernel bakes in: causal attn_mask, no padding, unit RMSNorm weights.
    assert np.array_equal(
        attn_mask.reshape(S, S), np.tril(np.ones((S, S), attn_mask.dtype)))
    assert padding_mask.all()
    assert np.all(qn_w == 1.0) and np.all(kn_w == 1.0)

    in_maps = _shard_inputs(q, k, v, cos, sin, proj_w)
    nc = _get_nc()
    res = run_bass_kernel_spmd(nc, in_maps, core_ids=list(range(N_CORES)))
    parts = np.stack([r["out"] for r in res.results])      # [8, B*S, E]
    full = parts.sum(axis=0, dtype=np.float32) + proj_b[None, :]
    return full.reshape(B, S, E).astype(np.float32)


# revision 20
# speedup vs baseline: 1.0145x; 1.0145x over previous
"""Trainium2 Bass kernel for nn_BaseMultiHeadAttention (B=2, S=2048, E=1024, H=16).

Sharding: tensor-parallel over heads — each of the 8 NeuronCores handles 2
heads for both batch elements (4 (b,h) jobs/core).  RMSNorm + RoPE + causal
attention run per-head on-device; the output projection is row-sharded
(each core contracts its 128 ctx features against proj_w), and the host
sums the 8 partial [B,S,E] outputs (the all-reduce) and adds the bias.

Device pipeline per core:
  Phase A (per job): load q/k [128, NT*64] (host pre-arranged
    partition-major so every DMA descriptor is a 4KB contiguous run),
    sum-of-squares -> sqrt(mean+eps) on ACT -> reciprocal on DVE, then
    batched RoPE (host permutes q/k features to de-interleave the rope
    pairs — a consistent permutation leaves q.k dot products unchanged,
    so every rope op is a contiguous full-width DVE op), then
    PE-transpose to qT/kT [64, S] (float32r) for the attention matmuls.
  Phase B (per b, q-chunk of 512, head): scoresT[k,q] = kT.T @ qT on PE in
    float32r (1 cyc/row), causal block-sparse (only k-blocks <= chunk end),
    triangular mask add on diagonal blocks (DVE), exp via ACT over
    [128,1024] PSUM groups (scale=D^-0.5 folded in; no max subtraction:
    RMSNorm bounds |scores*scale| <= ~8), ctx = p.T @ [v|1] accumulated in
    PSUM (the ones column yields softmax row-sums for free), rows scaled by
    1/sum on the PSUM->SBUF copy.  After both heads: PE-transpose ctx
    [s,128f] -> [128f,s], partial projection in float32r, DMA out.
  A and B share PSUM pools and are interleaved A0 A1 B0 A2 A3 B1 so the
  scheduler overlaps phase A of later jobs under phase B compute.
"""
import numpy as np

import bass_rust
import concourse.bass as bass
import concourse.mybir as mybir
import concourse.tile as tile
from concourse.bass_utils import run_bass_kernel_spmd
from concourse.masks import make_identity

B, S, E, H, D = 2, 2048, 1024, 16, 64
HD = D // 2
N_CORES = 8
HL = H // N_CORES          # 2 heads per core
NJ = B * HL                # 4 (b, h) jobs per core
NT = S // 128              # 16 s-tiles per job
NCH = S // 512             # 4 q-chunks per job
EPS = 1.1920928955078125e-07
SCALE = float(D) ** -0.5
NEG = -1e30
f32 = mybir.dt.float32
f32r = mybir.dt.float32r
ALU = mybir.AluOpType
ACTF = mybir.ActivationFunctionType

# float32r runs the PE at 1 cycle/row for moving dims >= 256 (vs fp32's 4).
F32R_SCORES = True
F32R_PROJ = True
# bf16 for the attention-weights @ v matmul (PE 1 cyc/row at N=65)
CTX_BF16 = False
bf16 = mybir.dt.bfloat16

_TC = tile.TileContext


def _legalize_waits(nc):
    """Split multi-wait sync_infos for this walrus build.

    This neuronxcc's codegen allows 1 sync wait per instruction (2 on
    EventSemaphore), while the Tile scheduler attaches all outstanding
    waits to one instruction.  Hoist the excess onto same-engine NoOps
    inserted immediately before the offending instruction — the engine
    executes its stream in order, so blocking semantics are identical.
    """
    uid = 0
    for f in nc.m.functions:
        for blk in f.blocks:
            insts = list(blk.instructions)
            out, changed = [], False
            for inst in insts:
                si = inst.sync_info
                cap = 2 if isinstance(inst, mybir.InstEventSemaphore) else 1
                if si is not None and len(si.on_wait) > cap:
                    changed = True
                    waits = list(si.on_wait)
                    for w in waits[:-cap]:
                        carrier = mybir.InstNoOp(
                            name=f"legwait-{uid}", engine=inst.engine,
                            ins=[], outs=[])
                        uid += 1
                        carrier.sync_info = bass_rust.SyncInfo(
                            on_wait=[w], on_update=[])
                        nc.register_instruction(carrier, overwrite=True)
                        out.append(carrier)
                    si.on_wait = waits[-cap:]
                    inst.sync_info = si
                out.append(inst)
            if changed:
                blk.instructions = out


def build_nc():
    nc = bass.Bass("TRN2", target_bir_lowering=False, debug=False)
    q_in = nc.dram_tensor("q", [NJ, 128, NT, D], f32, kind="ExternalInput")
    k_in = nc.dram_tensor("k", [NJ, 128, NT, D], f32, kind="ExternalInput")
    v_in = nc.dram_tensor("v", [NJ, 128, NT, D], f32, kind="ExternalInput")
    cos_in = nc.dram_tensor("cos", [128, NT, HD], f32, kind="ExternalInput")
    sin_in = nc.dram_tensor("sin", [128, NT, HD], f32, kind="ExternalInput")
    wt_in = nc.dram_tensor("wt", [128, E], f32, kind="ExternalInput")
    out = nc.dram_tensor("out", [B * S, E], f32, kind="ExternalOutput")

    with _TC(nc) as tc:
        with tc.tile_pool(name="const", bufs=1) as cp, \
             tc.tile_pool(name="pa", bufs=2) as pa, \
             tc.tile_pool(name="pb", bufs=2) as pb, \
             tc.tile_pool(name="pp", bufs=12) as pp, \
             tc.tile_pool(name="po", bufs=4) as po, \
             tc.tile_pool(name="ps_s", bufs=2, space="PSUM") as ps_s, \
             tc.tile_pool(name="ps_sm", bufs=2, space="PSUM") as ps_sm, \
             tc.tile_pool(name="ps_o", bufs=2, space="PSUM") as ps_o:
            ident = cp.tile([128, 128], f32)
            make_identity(nc, ident)
            eps_t = cp.tile([128, 1], f32)
            nc.vector.memset(eps_t, EPS)
            cos_sb = cp.tile([128, NT, HD], f32)
            sin_sb = cp.tile([128, NT, HD], f32)
            nc.sync.dma_start(out=cos_sb, in_=cos_in.ap())
            nc.sync.dma_start(out=sin_sb, in_=sin_in.ap())
            wt_sb = cp.tile([128, E], f32r if F32R_PROJ else f32)
            wt_raw = cp.tile([128, E], f32)
            nc.sync.dma_start(out=wt_raw, in_=wt_in.ap())
            nc.vector.tensor_copy(wt_sb, wt_raw)
            qT = cp.tile([64, NJ, S], f32r if F32R_SCORES else f32)
            kT = cp.tile([64, NJ, S], f32r if F32R_SCORES else f32)
            vsb = cp.tile([128, NJ, NT, D + 1],
                          bf16 if CTX_BF16 else f32)

            # ------------- Phase A: norm + rope + transpose (per job) -------
            def phase_a(j):
                for dstT, src in ((qT, q_in), (kT, k_in)):
                    raw = pa.tile([128, NT, D], f32, tag="raw", name="raw")
                    nc.sync.dma_start(out=raw, in_=src.ap()[j])
                    sq = pa.tile([128, NT, D], f32, tag="sq", name="sq")
                    nc.any.tensor_mul(sq, raw, raw)
                    ss = pa.tile([128, NT], f32, tag="ss", name="ss")
                    nc.vector.reduce_sum(ss, sq, axis=mybir.AxisListType.X)
                    rs = pa.tile([128, NT], f32, tag="rs", name="rs")
                    nc.scalar.activation(
                        out=rs, in_=ss, func=ACTF.Sqrt,
                        bias=eps_t, scale=1.0 / D,
                    )
                    nc.vector.reciprocal(out=rs, in_=rs)
                    # normalize: raw * rs (rs broadcast over d via step-0 AP)
                    rs_b = bass.AP(
                        tensor=rs.tensor, offset=rs.offset,
                        ap=[list(rs.ap[0]), list(rs.ap[1]), [0, D]])
                    xn = pa.tile([128, NT, D], f32, tag="xn", name="xn")
                    nc.gpsimd.tensor_mul(xn, raw, rs_b)
                    x1, x2 = xn[:, :, 0:HD], xn[:, :, HD:D]
                    rn_all = pa.tile([128, NT, D], f32, tag="rn", name="rn")
                    t1 = pa.tile([128, NT, HD], f32, tag="t1", name="t1")
                    t2 = pa.tile([128, NT, HD], f32, tag="t2", name="t2")
                    nc.any.tensor_mul(t1, x1, cos_sb)
                    nc.any.tensor_mul(t2, x2, sin_sb)
                    nc.any.tensor_sub(rn_all[:, :, 0:HD], t1, t2)
                    nc.any.tensor_mul(t1, x1, sin_sb)
                    nc.any.tensor_mul(t2, x2, cos_sb)
                    nc.any.tensor_add(rn_all[:, :, HD:D], t1, t2)
                    for tg in range(NT // 4):
                        ps_tr = ps_o.tile([64, 512], f32, tag="o",
                                          name="ps_tr")
                        for tt in range(4):
                            t = tg * 4 + tt
                            nc.tensor.transpose(
                                ps_tr[:, tt * 128:(tt + 1) * 128],
                                rn_all[:, t, :], ident)
                        nc.any.tensor_copy(
                            dstT[:, j, tg * 512:(tg + 1) * 512], ps_tr)
                # v is only needed by the ctx matmuls, well after q/k
                nc.gpsimd.memset(vsb[:, j, :, :], 1.0)
                if CTX_BF16:
                    vraw = pa.tile([128, NT, D], f32, tag="vraw", name="vraw")
                    nc.sync.dma_start(out=vraw, in_=v_in.ap()[j])
                    nc.vector.tensor_copy(vsb[:, j, :, 0:D], vraw)
                else:
                    nc.sync.dma_start(out=vsb[:, j, :, 0:D], in_=v_in.ap()[j])

            # ------------- Phase B: attention + projection (per batch) ------
            def phase_b(b):
                for c in range(NCH):
                    cpair = [pb.tile([128, HL * D], f32, tag=f"cpair{qb}",
                                     name=f"cpair{qb}")
                             for qb in range(4)]
                    for hl in range(HL):
                        j = b * HL + hl
                        ngrp = 2 * c + 2
                        ptiles = []
                        for g in range(ngrp):
                            sps = ps_s.tile([128, 1024], f32, tag="s",
                                            name="sps")
                            for u in range(2):
                                jj = 2 * g + u
                                nc.tensor.matmul(
                                    sps[:, u * 512:(u + 1) * 512],
                                    lhsT=kT[:, j, jj * 128:(jj + 1) * 128],
                                    rhs=qT[:, j, c * 512:(c + 1) * 512],
                                    start=True, stop=True,
                                )
                            pt = pp.tile([128, 1024],
                                         bf16 if CTX_BF16 else f32,
                                         tag="p", name="pt")
                            nc.scalar.activation(
                                out=pt, in_=sps, func=ACTF.Exp, scale=SCALE)
                            for u in range(2):
                                jj = 2 * g + u
                                if jj >= 4 * c:
                                    # diag block: zero the non-causal (q < k)
                                    # part of p on the idle gpsimd engine
                                    tl = jj - 4 * c
                                    sl = pt[:, u * 512 + tl * 128:
                                            u * 512 + (tl + 1) * 128]
                                    nc.gpsimd.affine_select(
                                        out=sl, in_=sl, compare_op=ALU.is_ge,
                                        fill=0.0, base=0, pattern=[[1, 128]],
                                        channel_multiplier=-1)
                            ptiles.append(pt)
                        ctx = ps_sm.tile([128, 4, D + 1], f32, tag="sm",
                                         name="ctx")
                        for qb in range(4):
                            i = 4 * c + qb
                            for jj in range(i + 1):
                                nc.tensor.matmul(
                                    ctx[:, qb, :],
                                    lhsT=ptiles[jj // 2][
                                        :, (jj % 2) * 512 + qb * 128:
                                           (jj % 2) * 512 + (qb + 1) * 128],
                                    rhs=vsb[:, j, jj, :],
                                    start=(jj == 0), stop=(jj == i),
                                )
                            rsum = pb.tile([128, 1], f32, tag="rsum",
                                           name="rsum")
                            nc.vector.reciprocal(out=rsum,
                                                 in_=ctx[:, qb, D:D + 1])
                            nc.vector.tensor_scalar_mul(
                                cpair[qb][:, hl * D:(hl + 1) * D],
                                ctx[:, qb, 0:D], rsum)
                    for qb in range(4):
                        i = 4 * c + qb
                        trp = ps_sm.tile([128, 128], f32, tag="sm", name="trp")
                        nc.tensor.transpose(trp, cpair[qb], ident)
                        ctxT = pb.tile([128, 128], f32r if F32R_PROJ else f32,
                                       tag="ctxT", name="ctxT")
                        nc.any.tensor_copy(ctxT, trp)
                        osb = po.tile([128, E], f32, tag="osb", name="osb")
                        for n in range(2):
                            ops_ = ps_o.tile([128, 512], f32, tag="o",
                                             name="ops")
                            nc.tensor.matmul(
                                ops_, lhsT=ctxT,
                                rhs=wt_sb[:, n * 512:(n + 1) * 512],
                                start=True, stop=True,
                            )
                            nc.any.tensor_copy(
                                osb[:, n * 512:(n + 1) * 512], ops_)
                        nc.gpsimd.dma_start(
                            out=out.ap()[b * S + i * 128:
                                         b * S + (i + 1) * 128, :],
                            in_=osb,
                        )

            phase_a(0)
            phase_a(1)
            phase_b(0)
            phase_a(2)
            phase_a(3)
            phase_b(1)
    _legalize_waits(nc)
    return nc


# even rope lanes first, then odd — see Phase A comment
_ROPE_PERM = np.concatenate([np.arange(0, D, 2), np.arange(1, D, 2)])


def _shard_inputs(q, k, v, cos, sin, proj_w):
    """Per-core input maps (host-side layout prep only — no module math)."""
    qh = q.reshape(B, S, H, D)
    kh = k.reshape(B, S, H, D)
    vh = v.reshape(B, S, H, D)
    # [S, HD] -> [128, NT, HD] partition-major
    cos_t = np.ascontiguousarray(
        cos.reshape(NT, 128, HD).transpose(1, 0, 2), np.float32)
    sin_t = np.ascontiguousarray(
        sin.reshape(NT, 128, HD).transpose(1, 0, 2), np.float32)
    maps = []
    for core in range(N_CORES):
        hs = slice(HL * core, HL * (core + 1))

        def tiles(x, permute):
            xs = x[:, :, hs, :].transpose(0, 2, 1, 3)  # [B, HL, S, D]
            if permute:
                xs = xs[..., _ROPE_PERM]
            # [NJ, NT, 128, D] -> [NJ, 128, NT, D] partition-major
            return np.ascontiguousarray(
                xs.reshape(NJ, NT, 128, D).transpose(0, 2, 1, 3), np.float32)

        wt_c = np.ascontiguousarray(
            proj_w[:, 128 * core:128 * (core + 1)].T, np.float32)
        maps.append({
            "q": tiles(qh, True), "k": tiles(kh, True),
            "v": tiles(vh, False),
            "cos": cos_t, "sin": sin_t, "wt": wt_c,
        })
    return maps


_NC_CACHE = []


def _get_nc():
    if not _NC_CACHE:
        _NC_CACHE.append(build_nc())
    return _NC_CACHE[0]


def kernel(q, k, v, attn_mask, padding_mask, qn_w, kn_w, proj_w, proj_b,
           cos, sin):
    q = np.asarray(q, np.float32)
    k = np.asarray(k, np.float32)
    v = np.asarray(v, np.float32)
    proj_w = np.asarray(proj_w, np.float32)
    proj_b = np.asarray(proj_b, np.float32)
    cos = np.asarray(cos, np.float32)
    sin = np.asarray(sin, np.float32)
    attn_mask = np.asarray(attn_mask)
    padding_mask = np.asarray(padding_mask)
    qn_w = np.asarray(qn_w, np.float32)
    kn_w = np.asarray(kn_w, np.float32)
    # The kernel bakes in: causal attn_mask, no padding, unit RMSNorm weights.
    assert np.array_equal(
        attn_mask.reshape(S, S), np.tril(np.ones((S, S), attn_mask.dtype)))
    assert padding_mask.all()
    assert np.all(qn_w == 1.0) and np.all(kn_w == 1.0)

    in_maps = _shard_inputs(q, k, v, cos, sin, proj_w)
    nc = _get_nc()
    res = run_bass_kernel_spmd(nc, in_maps, core_ids=list(range(N_CORES)))
    parts = np.stack([r["out"] for r in res.results])      # [8, B*S, E]
    full = parts.sum(axis=0, dtype=np.float32) + proj_b[None, :]
    return full.reshape(B, S, E).astype(np.float32)
